# revision 1
# baseline (speedup 1.0000x reference)
"""Damped electrostatics (charge+dipole+quadrupole, switched) over 3.2M edges
on 8 Trainium2 NeuronCores.

Strategy (data-parallel over edges):
  - Shard the [E]-indexed tensors across the 8 cores (400k edges each).
  - The per-atom tables are tiny (q/mu/Q ~5MB); the per-edge u/v records are
    resolved during host-side sharding into planar per-edge streams (device
    indirect-DMA gathers cost ~1.4us per 128 records on this HW -- ~9ms/core
    for 3.2M edges -- so data-dependent device gathers cannot approach the
    roofline; streaming planar operands can).
  - Edges are sorted by distance within each core (sharding is free to pick
    any edge->slot mapping; the inverse permutation is applied on unshard).
    With ascending d, all d<2 edges land in tile 0: only that tile evaluates
    the quintic switch / damped-Coulomb blend.  Tiles 1..6 use chi = 1/d
    exactly (switch_fn == 0 for d >= CUTOFF_SR).  Only the last tile needs
    the d <= CUTOFF mask (largest d sorts there).
  - The quadrupole contraction is pre-reduced per atom: with
    B = sym(Q) - (tr(Q)/3) I (traceless symmetrized), the per-edge term
    sum(traceless(outer(v,v)) * Q_v) / d^2 == v^T B_v v / d^2.
  - Device evaluates all per-edge floating-point math (switch function,
    damped Coulomb chi, dipole dots, quadrupole form) with DVE/ACT ops.
    GPSIMD is intentionally NOT used for elementwise work: it contends with
    DVE for SBUF ports (measured ~40% slowdown of concurrent DVE ops).
"""

import os
import sys

for _p in ("/opt/trn_rl_repo", "/root/.axon_site/_ro/trn_rl_repo"):
    if os.path.isdir(_p) and _p not in sys.path:
        sys.path.append(_p)

import numpy as np

import concourse.bass as bass
import concourse.mybir as mybir
import concourse.tile as tile
from concourse.bass_utils import run_bass_kernel_spmd

F32 = mybir.dt.float32
ALU = mybir.AluOpType
ACT = mybir.ActivationFunctionType

N_CORES = 8
N_ATOMS = 100000
N_EDGES = 3200000
E_CORE = N_EDGES // N_CORES          # 400000
P = 128
W_T = 400                            # tile width
N_TILES = 8
W_TOT = W_T * N_TILES                # 3200 columns; 409600 slots >= 400000
N_PLANES = 18   # d v0 v1 v2 | qu u0 u1 u2 | qv w0 w1 w2 | b00 b11 b22 c01 c02 c12

CUTOFF = 12.0
KEHALF = 7.199822675975274
SQRT2 = float(np.sqrt(2.0))

_MAX_WAITS = 1  # this walrus build allows only 1 sync wait on some instruction types


def _split_sync_waits(nc):
    """Walrus here fails codegen ("Too many sync wait commands") for any
    instruction carrying more than _MAX_WAITS semaphore waits. Move excess
    waits onto same-engine NOPs inserted immediately before the instruction:
    the sequencer executes waits in program order, so this is equivalent."""
    import bass_rust

    counter = [0]
    for fn in nc.m.functions:
        for bb in fn.blocks:
            insts = list(bb.instructions)
            out = []
            changed = False
            for inst in insts:
                si = inst.sync_info
                waits = list(si.on_wait) if (si and si.on_wait) else []
                if len(waits) > _MAX_WAITS:
                    changed = True
                    head, rest = waits[:-_MAX_WAITS], waits[-_MAX_WAITS:]
                    for i in range(0, len(head), _MAX_WAITS):
                        counter[0] += 1
                        nop = bass_rust.InstNoOp(
                            name=f"I-waitsplit-{counter[0]}", ins=[], outs=[]
                        )
                        nop.engine = inst.engine
                        nop.sync_info = mybir.SyncInfo(
                            on_wait=head[i:i + _MAX_WAITS], on_update=[]
                        )
                        out.append(nop)
                    si.on_wait = rest
                out.append(inst)
            if changed:
                bb.instructions = out


def _build_module():
    nc = bass.Bass()

    # host pre-interleaves planes per tile: [P, N_TILES, N_PLANES, W_T]
    x_in = nc.dram_tensor(
        "x", [P, N_TILES, N_PLANES, W_T], F32, kind="ExternalInput"
    )
    out = nc.dram_tensor("out", [P, W_TOT], F32, kind="ExternalOutput")

    with tile.TileContext(nc) as tc:
        with (
            tc.tile_pool(name="io", bufs=3) as io_pool,
            tc.tile_pool(name="scr", bufs=2) as scr_pool,
        ):
            for it in range(N_TILES):
                slow = it == 0          # only tile 0 holds d < 2 edges
                masked = it == N_TILES - 1   # only last tile holds d > CUTOFF

                sl = slice(it * W_T, (it + 1) * W_T)
                # geometry planes land first so the chi chain starts while
                # the (larger) atom-feature block is still in flight
                xta = io_pool.tile([P, 4 * W_T], F32, tag="xta")
                nc.sync.dma_start(
                    out=xta[:],
                    in_=x_in[:, it, 0:4].rearrange("p k w -> p (k w)"),
                )
                xtb = io_pool.tile([P, 14 * W_T], F32, tag="xtb")
                nc.sync.dma_start(
                    out=xtb[:],
                    in_=x_in[:, it, 4:N_PLANES].rearrange("p k w -> p (k w)"),
                )

                def pl(k):
                    if k < 4:
                        return xta[:, k * W_T:(k + 1) * W_T]
                    k -= 4
                    return xtb[:, k * W_T:(k + 1) * W_T]

                d = pl(0)
                v0, v1, v2 = pl(1), pl(2), pl(3)
                qu, u0, u1, u2 = pl(4), pl(5), pl(6), pl(7)
                qv, w0, w1, w2 = pl(8), pl(9), pl(10), pl(11)
                b00, b11, b22 = pl(12), pl(13), pl(14)
                c01, c02, c12 = pl(15), pl(16), pl(17)

                def scr(tag):
                    return scr_pool.tile([P, W_T], F32, tag=tag, name=tag)

                if slow:
                    # full chi(d) = sw/sqrt(d^2+1) + (1-sw)/d
                    # one reciprocal: rc = 1/(d*dd) -> 1/d = rc*dd, 1/dd = rc*d
                    sq = scr("sq")
                    nc.scalar.activation(sq[:], d, ACT.Square)
                    dd = scr("dd")        # sqrt(d^2+1)
                    nc.scalar.activation(dd[:], sq[:], ACT.Sqrt, bias=1.0)
                    prod = scr("prod")
                    nc.vector.tensor_tensor(prod[:], d, dd[:], ALU.mult)
                    rc = scr("rc")
                    nc.vector.reciprocal(out=rc[:], in_=prod[:])
                    r = scr("r")          # 1/d
                    nc.vector.tensor_tensor(r[:], rc[:], dd[:], ALU.mult)
                    ri = scr("ri")        # 1/sqrt(d^2+1)
                    nc.vector.tensor_tensor(ri[:], rc[:], d, ALU.mult)

                    x = scr("x")          # clip(d/2, 0, 1)
                    nc.vector.tensor_scalar(x[:], d, 0.5, 1.0, ALU.mult, ALU.min)
                    h1 = scr("h1")        # 15 - 6x
                    nc.vector.tensor_scalar(
                        h1[:], x[:], -6.0, 15.0, ALU.mult, ALU.add
                    )
                    h2 = scr("h2")        # x*(15-6x)
                    nc.vector.tensor_tensor(h2[:], h1[:], x[:], ALU.mult)
                    x2 = scr("x2")
                    nc.scalar.activation(x2[:], x[:], ACT.Square)
                    x3 = scr("x3")
                    nc.vector.tensor_tensor(x3[:], x2[:], x[:], ALU.mult)
                    swm1 = scr("swm1")    # sw - 1 = (h2 - 10)*x^3
                    nc.vector.scalar_tensor_tensor(
                        swm1[:], h2[:], -10.0, x3[:], ALU.add, ALU.mult
                    )
                    rdif = scr("rdif")    # ri - r
                    nc.vector.tensor_tensor(rdif[:], ri[:], r[:], ALU.subtract)
                    chi = scr("chi")      # ri + (sw-1)*(ri-r)
                    nc.vector.tensor_tensor(chi[:], swm1[:], rdif[:], ALU.mult)
                    nc.vector.tensor_tensor(chi[:], chi[:], ri[:], ALU.add)

                    chi2m = scr("chi2m")  # 2*chi^2
                    nc.scalar.activation(chi2m[:], chi[:], ACT.Square, scale=SQRT2)
                    t3 = scr("t3")        # chi^3 = 0.5*chi2m*chi
                    nc.vector.scalar_tensor_tensor(
                        t3[:], chi2m[:], 0.5, chi[:], ALU.mult, ALU.mult
                    )
                    r2 = scr("r2")        # 1/d^2
                    nc.scalar.activation(r2[:], r[:], ACT.Square)
                    c2 = scr("c2")        # 2*chi^2/d  (term1 factor / KEHALF)
                    nc.vector.tensor_tensor(c2[:], chi2m[:], r[:], ALU.mult)
                    t5 = scr("t5")        # chi^3/d^2
                    nc.vector.tensor_tensor(t5[:], t3[:], r2[:], ALU.mult)
                else:
                    # d >= 2 -> sw == 0 -> chi = 1/d exactly.
                    # Power ladder via ACT Ln/Exp; 1/d Newton-polished (the
                    # charge term is dominant); r^3, r^5 raw table (~1.3e-4,
                    # feeds only the smaller dipole/quadrupole factors).
                    L = scr("L")
                    nc.scalar.activation(L[:], d, ACT.Ln)
                    chi = scr("chi")      # 1/d from the Exp table (~4e-5 rel)
                    nc.scalar.activation(chi[:], L[:], ACT.Exp, scale=-1.0)
                    r = chi
                    t3 = scr("t3")        # 1/d^3 (= chi^3)
                    nc.scalar.activation(t3[:], L[:], ACT.Exp, scale=-3.0)
                    t5 = scr("t5")        # 1/d^5 (= chi^3/d^2)
                    nc.scalar.activation(t5[:], L[:], ACT.Exp, scale=-5.0)
                    c2 = t3               # term1 uses 2*KE*t3 via the stt scalar

                # --- charge term: e = KE*(qu*qv)*chi ---
                e = scr("e")
                nc.vector.tensor_tensor(e[:], qu, qv, ALU.mult)
                nc.vector.scalar_tensor_tensor(
                    e[:], e[:], KEHALF, chi[:], ALU.mult, ALU.mult
                )

                # --- dipole dots (raw v; 1/d powers folded into c2/t5) ---
                tmp = scr("tmp")
                sv = scr("sv")        # v . mu_v
                nc.vector.tensor_tensor(sv[:], v0, w0, ALU.mult)
                nc.vector.tensor_tensor(tmp[:], v1, w1, ALU.mult)
                nc.vector.tensor_tensor(sv[:], sv[:], tmp[:], ALU.add)
                nc.vector.tensor_tensor(tmp[:], v2, w2, ALU.mult)
                nc.vector.tensor_tensor(sv[:], sv[:], tmp[:], ALU.add)
                su = scr("su")        # v . mu_u
                nc.vector.tensor_tensor(su[:], v0, u0, ALU.mult)
                nc.vector.tensor_tensor(tmp[:], v1, u1, ALU.mult)
                nc.vector.tensor_tensor(su[:], su[:], tmp[:], ALU.add)
                nc.vector.tensor_tensor(tmp[:], v2, u2, ALU.mult)
                nc.vector.tensor_tensor(su[:], su[:], tmp[:], ALU.add)
                uvd = scr("uvd")      # mu_u . mu_v
                nc.vector.tensor_tensor(uvd[:], u0, w0, ALU.mult)
                nc.vector.tensor_tensor(tmp[:], u1, w1, ALU.mult)
                nc.vector.tensor_tensor(uvd[:], uvd[:], tmp[:], ALU.add)
                nc.vector.tensor_tensor(tmp[:], u2, w2, ALU.mult)
                nc.vector.tensor_tensor(uvd[:], uvd[:], tmp[:], ALU.add)

                # --- quadrupole form: wq = qu * v^T B v ---
                v00, v11, v22 = scr("v00"), scr("v11"), scr("v22")
                nc.scalar.activation(v00[:], v0, ACT.Square)
                nc.scalar.activation(v11[:], v1, ACT.Square)
                nc.scalar.activation(v22[:], v2, ACT.Square)
                wq = scr("wq")
                nc.vector.tensor_tensor(wq[:], v00[:], b00, ALU.mult)
                nc.vector.tensor_tensor(tmp[:], v11[:], b11, ALU.mult)
                nc.vector.tensor_tensor(wq[:], wq[:], tmp[:], ALU.add)
                nc.vector.tensor_tensor(tmp[:], v22[:], b22, ALU.mult)
                nc.vector.tensor_tensor(wq[:], wq[:], tmp[:], ALU.add)
                v01 = scr("v01")
                nc.vector.tensor_tensor(v01[:], v0, v1, ALU.mult)
                nc.vector.tensor_tensor(tmp[:], v01[:], c01, ALU.mult)
                nc.vector.tensor_tensor(wq[:], wq[:], tmp[:], ALU.add)
                nc.vector.tensor_tensor(v01[:], v0, v2, ALU.mult)
                nc.vector.tensor_tensor(tmp[:], v01[:], c02, ALU.mult)
                nc.vector.tensor_tensor(wq[:], wq[:], tmp[:], ALU.add)
                nc.vector.tensor_tensor(v01[:], v1, v2, ALU.mult)
                nc.vector.tensor_tensor(tmp[:], v01[:], c12, ALU.mult)
                nc.vector.tensor_tensor(wq[:], wq[:], tmp[:], ALU.add)
                nc.vector.tensor_tensor(wq[:], wq[:], qu, ALU.mult)

                # term1: e += KE*(qu*sv) * (2 chi^2 / d)   [c2 = 2chi^2/d]
                t1 = scr("t1")
                nc.vector.tensor_tensor(t1[:], qu, sv[:], ALU.mult)
                nc.vector.scalar_tensor_tensor(
                    t1[:], t1[:], KEHALF if slow else 2.0 * KEHALF, c2[:],
                    ALU.mult, ALU.mult
                )
                nc.vector.tensor_tensor(e[:], e[:], t1[:], ALU.add)
                # term2a: e += KE*(mu_u.mu_v) * chi^3
                m1 = scr("m1")
                nc.vector.scalar_tensor_tensor(
                    m1[:], uvd[:], KEHALF, t3[:], ALU.mult, ALU.mult
                )
                nc.vector.tensor_tensor(e[:], e[:], m1[:], ALU.add)
                # term2b+3: e += KE*(qu*v^T B v - 3*sv*su) * chi^3/d^2
                p = scr("p")
                nc.vector.tensor_tensor(p[:], sv[:], su[:], ALU.mult)
                m2 = scr("m2")
                nc.vector.scalar_tensor_tensor(
                    m2[:], p[:], -3.0, wq[:], ALU.mult, ALU.add
                )
                nc.vector.scalar_tensor_tensor(
                    m2[:], m2[:], KEHALF, t5[:], ALU.mult, ALU.mult
                )
                nc.vector.tensor_tensor(e[:], e[:], m2[:], ALU.add)

                if masked:
                    # zero edges with d > CUTOFF; largest d sorts here
                    mask = scr("mask")
                    nc.vector.tensor_scalar(
                        mask[:], d, CUTOFF, None, ALU.is_le
                    )
                    res = io_pool.tile([P, W_T], F32, tag="res")
                    nc.vector.tensor_tensor(res[:], e[:], mask[:], ALU.mult)
                else:
                    res = e

                nc.sync.dma_start(out=out[:, sl], in_=res[:])

    return nc


def _prep_inputs(distances_uv, vectors_uv, atomic_charges, atomic_dipoles,
                 atomic_quadrupoles, idx_u, idx_v):
    d = np.ascontiguousarray(np.asarray(distances_uv, dtype=np.float32))
    vec = np.ascontiguousarray(np.asarray(vectors_uv, dtype=np.float32))
    q = np.asarray(atomic_charges, dtype=np.float32)
    mu = np.asarray(atomic_dipoles, dtype=np.float32)
    Q = np.asarray(atomic_quadrupoles, dtype=np.float32)
    iu = np.asarray(idx_u, dtype=np.int64)
    iv = np.asarray(idx_v, dtype=np.int64)

    # traceless symmetrized quadrupole, off-diagonals doubled
    B = 0.5 * (Q + np.swapaxes(Q, 1, 2))
    tr3 = (np.trace(Q, axis1=1, axis2=2) / 3.0).astype(np.float32)
    bt = np.empty((N_ATOMS, 6), dtype=np.float32)
    bt[:, 0] = B[:, 0, 0] - tr3
    bt[:, 1] = B[:, 1, 1] - tr3
    bt[:, 2] = B[:, 2, 2] - tr3
    bt[:, 3] = 2.0 * B[:, 0, 1]
    bt[:, 4] = 2.0 * B[:, 0, 2]
    bt[:, 5] = 2.0 * B[:, 1, 2]

    in_maps = []
    orders = []
    for c in range(N_CORES):
        s = slice(c * E_CORE, (c + 1) * E_CORE)
        dc = d[s]
        order = np.argsort(dc, kind="stable")
        orders.append(order)
        n_lt2 = int((dc < 2.0).sum())
        assert n_lt2 <= P * W_T, (
            f"core {c}: {n_lt2} edges with d<2 exceed the slow tile"
        )

        iuc = iu[s][order]
        ivc = iv[s][order]
        planes = np.zeros((N_PLANES, P * W_TOT), dtype=np.float32)
        planes[0, :E_CORE] = dc[order]
        planes[0, E_CORE:] = 1.0                       # pad: harmless d
        vc = vec[s][order]
        planes[1, :E_CORE] = vc[:, 0]
        planes[2, :E_CORE] = vc[:, 1]
        planes[3, :E_CORE] = vc[:, 2]
        planes[4, :E_CORE] = q[iuc]
        muu = mu[iuc]
        planes[5, :E_CORE] = muu[:, 0]
        planes[6, :E_CORE] = muu[:, 1]
        planes[7, :E_CORE] = muu[:, 2]
        planes[8, :E_CORE] = q[ivc]
        muv = mu[ivc]
        planes[9, :E_CORE] = muv[:, 0]
        planes[10, :E_CORE] = muv[:, 1]
        planes[11, :E_CORE] = muv[:, 2]
        bv = bt[ivc]
        for k in range(6):
            planes[12 + k, :E_CORE] = bv[:, k]

        # slot k -> (p = k % P, w = k // P): column-major so ascending d
        # fills tile 0 first.  planes view [N_PLANES, W_TOT, P] -> device
        # layout [P, N_TILES, N_PLANES, W_T].
        pv = planes.reshape(N_PLANES, W_TOT, P)        # [k, w, p]
        xi = np.ascontiguousarray(
            pv.reshape(N_PLANES, N_TILES, W_T, P).transpose(3, 1, 0, 2)
        )
        in_maps.append({"x": xi})
    return in_maps, orders


def _run(inputs, trace=False, tmpdir=None):
    in_maps, orders = _prep_inputs(**inputs)
    nc = _build_module()
    _split_sync_waits(nc)
    res = run_bass_kernel_spmd(
        nc, in_maps, list(range(N_CORES)), trace=trace, tmpdir=tmpdir
    )
    full = np.empty(N_EDGES, dtype=np.float32)
    for c in range(N_CORES):
        o = res.results[c]["out"]                      # [P, W_TOT]
        slots = o.T.reshape(-1)[:E_CORE]               # column-major slots
        full[c * E_CORE + orders[c]] = slots
    return full, res


def kernel(**inputs):
    full, _ = _run(inputs, trace=False)
    return full



# revision 8
# speedup vs baseline: 1.9271x; 1.9271x over previous
"""Damped electrostatics (charge+dipole+quadrupole, switched) over 3.2M edges
on 8 Trainium2 NeuronCores.

Strategy (data-parallel over edges):
  - Shard the [E]-indexed tensors across the 8 cores (400k edges each).
  - Per-edge u/v atom records are resolved during host-side sharding into
    planar per-edge streams (device indirect-DMA gathers cost ~1.4us per 128
    records on this HW -- far off the roofline; streaming planar operands
    is the only way to feed the DVE at rate).
  - All device math runs in fp16: every DVE tensor_tensor ALU op qualifies
    for the 2x_1p perf mode (2-byte packed operands -> 0.5 cycle/elem) and
    DMA bytes halve.  Tolerance is 2e-2 vs a measured fp32 error of ~3e-6,
    so fp16 (~1e-3 elementwise) has ample margin.  Ln/Exp intermediates
    (L = ln d) stay fp32 on the ACT engine: Exp(-5L) amplifies input error
    5x and fp16 quantization of L would cost ~1% there.
  - Edges are sorted by distance within each core (the slot->edge mapping is
    inverted on unshard).  With ascending d all d<2 edges land in tile 0:
    only that tile evaluates the quintic switch / damped-Coulomb blend
    (exact for d>=2 too, so tile-0 overflow slots are still correct).
    Tiles 1..4 use chi = 1/d via the ACT Ln/Exp tables.  Only the last tile
    needs the d <= CUTOFF mask (largest d sorts there).
  - Constant folding: sqrt(KEHALF) is folded into the per-atom charge/
    dipole/quadrupole tables on the host, so every per-edge u*v product
    carries KEHALF automatically.  The charge-term 2x (from qu2 = 2*qu,
    needed by the dipole term) is cancelled by folding ln(0.5) into the
    Exp biases of r1 and r5.  The quadrupole table is pre-reduced per atom:
    B = sym(Q) - (tr(Q)/3) I with off-diagonals doubled, so the per-edge
    quadrupole contraction is v^T B v (6 products).
  - Vector-engine work is issued as wide block ops over contiguous 3-plane
    groups ([128, 3W] per instruction) to amortize per-instruction
    overhead; tiles are uneven (one 400-wide switch tile, four 700-wide
    fast tiles) so the expensive switch path only covers the columns that
    need it.  GPSIMD is intentionally NOT used: it contends with DVE for
    SBUF ports (measured ~40% slowdown of concurrent DVE ops).
"""

import os
import sys

for _p in ("/opt/trn_rl_repo", "/root/.axon_site/_ro/trn_rl_repo"):
    if os.path.isdir(_p) and _p not in sys.path:
        sys.path.append(_p)

import numpy as np

import concourse.bass as bass
import concourse.mybir as mybir
import concourse.tile as tile
from concourse.bass_utils import run_bass_kernel_spmd

F32 = mybir.dt.float32
F16 = mybir.dt.float16
ALU = mybir.AluOpType
ACT = mybir.ActivationFunctionType

N_CORES = 8
N_ATOMS = 100000
N_EDGES = 3200000
E_CORE = N_EDGES // N_CORES          # 400000
P = 128
W_TOT = 3200                         # 409600 slots >= 400000
TILE_W = [400, 700, 700, 700, 700]   # tile 0 = switch region (d < 2)
N_PLANES = 18
LN_HALF = -0.6931471805599453

CUTOFF = 12.0
KEHALF = 7.199822675975274

_MAX_WAITS = 1  # this walrus build allows only 1 sync wait on some instruction types


def _split_sync_waits(nc):
    """Walrus here fails codegen ("Too many sync wait commands") for any
    instruction carrying more than _MAX_WAITS semaphore waits. Move excess
    waits onto same-engine NOPs inserted immediately before the instruction:
    the sequencer executes waits in program order, so this is equivalent."""
    import bass_rust

    counter = [0]
    for fn in nc.m.functions:
        for bb in fn.blocks:
            insts = list(bb.instructions)
            out = []
            changed = False
            for inst in insts:
                si = inst.sync_info
                waits = list(si.on_wait) if (si and si.on_wait) else []
                if len(waits) > _MAX_WAITS:
                    changed = True
                    head, rest = waits[:-_MAX_WAITS], waits[-_MAX_WAITS:]
                    for i in range(0, len(head), _MAX_WAITS):
                        counter[0] += 1
                        nop = bass_rust.InstNoOp(
                            name=f"I-waitsplit-{counter[0]}", ins=[], outs=[]
                        )
                        nop.engine = inst.engine
                        nop.sync_info = mybir.SyncInfo(
                            on_wait=head[i:i + _MAX_WAITS], on_update=[]
                        )
                        out.append(nop)
                    si.on_wait = rest
                out.append(inst)
            if changed:
                bb.instructions = out


def _register_const(nc, value, dtype=F32):
    t = nc.alloc_sbuf_tensor(f"const-{dtype.name}-{value}", [128, 1], dtype)
    nc.gpsimd.memset(t.ap(), value)
    nc.const_aps.aps[(dtype, value)] = t.ap()


def _build_module():
    nc = bass.Bass()
    _register_const(nc, LN_HALF)
    nc.all_engine_barrier()

    # host packs per tile: [P, sum_t(18*W_t)] fp16, planes contiguous per tile
    total_cols = N_PLANES * W_TOT
    x_in = nc.dram_tensor("x", [P, total_cols], F16, kind="ExternalInput")
    out = nc.dram_tensor("out", [P, W_TOT], F16, kind="ExternalOutput")

    with tile.TileContext(nc) as tc:
        with (
            tc.tile_pool(name="io", bufs=3) as io_pool,
            tc.tile_pool(name="scr", bufs=2) as scr_pool,
        ):
            x_off = 0
            o_off = 0
            for it, W in enumerate(TILE_W):
                slow = it == 0                    # only tile 0 holds d < 2
                masked = it == len(TILE_W) - 1    # only last tile holds d > CUTOFF
                W3 = 3 * W

                xt = io_pool.tile([P, N_PLANES * W], F16, tag="xt")
                nc.sync.dma_start(
                    out=xt[:], in_=x_in[:, x_off:x_off + N_PLANES * W]
                )
                x_off += N_PLANES * W

                def pl(k, n=1):
                    return xt[:, k * W:(k + n) * W]

                d = pl(0)
                vblk = pl(1, 3)
                wblk = pl(4, 3)
                ublk = pl(7, 3)
                qv = pl(10)
                qu2 = pl(11)
                bblk = pl(12, 3)
                cblk = pl(15, 3)
                v0, v1, v2 = pl(1), pl(2), pl(3)

                def scr(tag, w=W, dt=F16):
                    return scr_pool.tile([P, w], dt, tag=tag, name=tag)

                # --- chi powers on the ACT engine ---------------------------
                # r1 = 0.5*chi ; rA = chi^2/d (term1) ; rB = chi^3 ;
                # r5 = 0.5*chi^3/d^2.  Fast path (chi = 1/d): rA == rB.
                r1 = scr("r1")
                r3 = scr("r3")
                r5 = scr("r5")
                if slow:
                    # chi(d) = sw/sqrt(d^2+1) + (1-sw)/d, quintic sw
                    sq = scr("sq", dt=F32)
                    nc.scalar.activation(sq[:], d, ACT.Square)
                    L2 = scr("L2", dt=F32)
                    nc.scalar.activation(L2[:], sq[:], ACT.Ln, bias=1.0)
                    ri = scr("ri")        # 0.5/sqrt(d^2+1)
                    nc.scalar.activation(
                        ri[:], L2[:], ACT.Exp, bias=LN_HALF, scale=-0.5
                    )
                    L = scr("L", dt=F32)
                    nc.scalar.activation(L[:], d, ACT.Ln)
                    r = scr("r")          # 0.5/d
                    nc.scalar.activation(
                        r[:], L[:], ACT.Exp, bias=LN_HALF, scale=-1.0
                    )
                    x = scr("x")          # clip(d/2, 0, 1)
                    nc.vector.tensor_scalar(x[:], d, 0.5, 1.0, ALU.mult, ALU.min)
                    h = scr("h")          # 15 - 6x
                    nc.vector.tensor_scalar(
                        h[:], x[:], -6.0, 15.0, ALU.mult, ALU.add
                    )
                    x2 = scr("x2")
                    nc.scalar.activation(x2[:], x[:], ACT.Square)
                    x3 = scr("x3")
                    nc.vector.tensor_tensor(x3[:], x2[:], x[:], ALU.mult)
                    h2 = scr("h2")        # x*(15-6x)
                    nc.vector.tensor_tensor(h2[:], h[:], x[:], ALU.mult)
                    swm1 = scr("swm1")    # sw - 1 = (h2 - 10)*x^3
                    nc.vector.scalar_tensor_tensor(
                        swm1[:], h2[:], -10.0, x3[:], ALU.add, ALU.mult
                    )
                    rdif = scr("rdif")    # ri - r
                    nc.vector.tensor_tensor(rdif[:], ri[:], r[:], ALU.subtract)
                    # r1 = ri + (sw-1)*(ri-r) = 0.5*chi
                    nc.vector.tensor_tensor(r1[:], swm1[:], rdif[:], ALU.mult)
                    nc.vector.tensor_tensor(r1[:], r1[:], ri[:], ALU.add)
                    c2 = scr("c2")        # chi^2 = (2*r1)^2
                    nc.scalar.activation(c2[:], r1[:], ACT.Square, scale=2.0)
                    rA = scr("rA")        # chi^2/d = c2 * 2 * (0.5/d)
                    nc.vector.scalar_tensor_tensor(
                        rA[:], c2[:], 2.0, r[:], ALU.mult, ALU.mult
                    )
                    # rB = chi^3 = 2*c2*r1
                    nc.vector.scalar_tensor_tensor(
                        r3[:], c2[:], 2.0, r1[:], ALU.mult, ALU.mult
                    )
                    rr2 = scr("rr2")      # 1/d^2 = (2*r)^2
                    nc.scalar.activation(rr2[:], r[:], ACT.Square, scale=2.0)
                    # r5 = 0.5*chi^3/d^2
                    nc.vector.scalar_tensor_tensor(
                        r5[:], r3[:], 0.5, rr2[:], ALU.mult, ALU.mult
                    )
                else:
                    # d >= 2 -> sw == 0 -> chi = 1/d exactly (ACT Ln/Exp
                    # tables, ~1.3e-4 rel; tolerance is 2e-2)
                    L = scr("L", dt=F32)
                    nc.scalar.activation(L[:], d, ACT.Ln)
                    nc.scalar.activation(
                        r1[:], L[:], ACT.Exp, bias=LN_HALF, scale=-1.0
                    )
                    nc.scalar.activation(r3[:], L[:], ACT.Exp, scale=-3.0)
                    nc.scalar.activation(
                        r5[:], L[:], ACT.Exp, bias=LN_HALF, scale=-5.0
                    )

                # --- dot products as 3-plane block ops ----------------------
                pw = scr("pw", W3)        # v .* mu_v
                nc.vector.tensor_tensor(pw[:], vblk, wblk, ALU.mult)
                pu = scr("pu", W3)        # v .* mu_u
                nc.vector.tensor_tensor(pu[:], vblk, ublk, ALU.mult)
                pm = scr("pm", W3)        # mu_u .* mu_v
                nc.vector.tensor_tensor(pm[:], ublk, wblk, ALU.mult)
                sv = scr("sv")            # KE * (v . mu_v)
                nc.vector.tensor_tensor(sv[:], pw[:, 0:W], pw[:, W:2 * W], ALU.add)
                nc.vector.tensor_tensor(sv[:], sv[:], pw[:, 2 * W:W3], ALU.add)
                su = pu[:, 0:W]           # v . mu_u (sqrt(KE) scale)
                nc.vector.tensor_tensor(su, pu[:, 0:W], pu[:, W:2 * W], ALU.add)
                nc.vector.tensor_tensor(su, su, pu[:, 2 * W:W3], ALU.add)
                M = pm[:, 0:W]            # KE * (mu_u . mu_v)
                nc.vector.tensor_tensor(M, pm[:, 0:W], pm[:, W:2 * W], ALU.add)
                nc.vector.tensor_tensor(M, M, pm[:, 2 * W:W3], ALU.add)

                # --- quadrupole form wq = v^T B v ---------------------------
                vsq = scr("vsq", W3)
                nc.scalar.activation(vsq[:], vblk, ACT.Square)
                vp = scr("vp", W3)        # v0v1 | v0v2 | v1v2
                nc.vector.tensor_tensor(vp[:, 0:W], v0, v1, ALU.mult)
                nc.vector.tensor_tensor(vp[:, W:2 * W], v0, v2, ALU.mult)
                nc.vector.tensor_tensor(vp[:, 2 * W:W3], v1, v2, ALU.mult)
                nc.vector.tensor_tensor(vsq[:], vsq[:], bblk, ALU.mult)
                nc.vector.tensor_tensor(vp[:], vp[:], cblk, ALU.mult)
                nc.vector.tensor_tensor(vsq[:], vsq[:], vp[:], ALU.add)
                wq = vsq[:, 0:W]
                nc.vector.tensor_tensor(wq, vsq[:, 0:W], vsq[:, W:2 * W], ALU.add)
                nc.vector.tensor_tensor(wq, wq, vsq[:, 2 * W:W3], ALU.add)

                # --- assemble ----------------------------------------------
                # E = cq*r1 + (qu2*sv)*rA + M*rB + (qu2*wq - 6 sv su)*r5
                # (d > CUTOFF edges have multipole planes zeroed on the host,
                # so their E is exactly 0 without a mask op)
                e = scr("e")
                nc.vector.tensor_tensor(e[:], qu2, qv, ALU.mult)
                nc.vector.tensor_tensor(e[:], e[:], r1[:], ALU.mult)
                t = scr("t")
                nc.vector.tensor_tensor(t[:], qu2, sv[:], ALU.mult)
                if slow:
                    nc.vector.tensor_tensor(t[:], t[:], rA[:], ALU.mult)
                    nc.vector.tensor_tensor(e[:], e[:], t[:], ALU.add)
                    nc.vector.tensor_tensor(t[:], M, r3[:], ALU.mult)
                    nc.vector.tensor_tensor(e[:], e[:], t[:], ALU.add)
                else:
                    nc.vector.tensor_tensor(t[:], t[:], M, ALU.add)
                    nc.vector.tensor_tensor(t[:], t[:], r3[:], ALU.mult)
                    nc.vector.tensor_tensor(e[:], e[:], t[:], ALU.add)
                nc.vector.tensor_tensor(wq, wq, qu2, ALU.mult)
                p = scr("p")
                nc.vector.tensor_tensor(p[:], sv[:], su, ALU.mult)
                nc.vector.scalar_tensor_tensor(
                    p[:], p[:], -6.0, wq, ALU.mult, ALU.add
                )
                nc.vector.tensor_tensor(p[:], p[:], r5[:], ALU.mult)

                res = io_pool.tile([P, W], F16, tag="res")
                nc.vector.tensor_tensor(res[:], e[:], p[:], ALU.add)

                nc.sync.dma_start(out=out[:, o_off:o_off + W], in_=res[:])
                o_off += W

    return nc


def _prep_inputs(distances_uv, vectors_uv, atomic_charges, atomic_dipoles,
                 atomic_quadrupoles, idx_u, idx_v):
    d = np.ascontiguousarray(np.asarray(distances_uv, dtype=np.float32))
    vec = np.ascontiguousarray(np.asarray(vectors_uv, dtype=np.float32))
    q = np.asarray(atomic_charges, dtype=np.float32)
    mu = np.asarray(atomic_dipoles, dtype=np.float32)
    Q = np.asarray(atomic_quadrupoles, dtype=np.float32)
    iu = np.asarray(idx_u, dtype=np.int64)
    iv = np.asarray(idx_v, dtype=np.int64)

    rke = np.float32(np.sqrt(KEHALF))
    qs = rke * q                      # sqrt(KE) * q
    qs2 = 2.0 * qs                    # 2 sqrt(KE) * q
    mus = rke * mu                    # sqrt(KE) * mu

    # traceless symmetrized quadrupole, off-diagonals doubled, sqrt(KE) scaled
    B = 0.5 * (Q + np.swapaxes(Q, 1, 2))
    tr3 = (np.trace(Q, axis1=1, axis2=2) / 3.0).astype(np.float32)
    bt = np.empty((N_ATOMS, 6), dtype=np.float32)
    bt[:, 0] = rke * (B[:, 0, 0] - tr3)
    bt[:, 1] = rke * (B[:, 1, 1] - tr3)
    bt[:, 2] = rke * (B[:, 2, 2] - tr3)
    bt[:, 3] = rke * 2.0 * B[:, 0, 1]
    bt[:, 4] = rke * 2.0 * B[:, 0, 2]
    bt[:, 5] = rke * 2.0 * B[:, 1, 2]

    n_slots = P * W_TOT
    in_maps = []
    orders = []
    for c in range(N_CORES):
        s = slice(c * E_CORE, (c + 1) * E_CORE)
        dc = d[s]
        order = np.argsort(dc, kind="stable")
        orders.append(order)
        n_lt2 = int((dc < 2.0).sum())
        assert n_lt2 <= P * TILE_W[0], (
            f"core {c}: {n_lt2} edges with d<2 exceed the switch tile"
        )

        iuc = iu[s][order]
        ivc = iv[s][order]
        ds = dc[order]
        # edges beyond the cutoff sort to the tail: zero their multipole
        # planes so every term vanishes exactly (replaces the device mask)
        n_live = int(np.searchsorted(ds, np.float32(CUTOFF), side="right"))
        planes = np.zeros((N_PLANES, n_slots), dtype=np.float32)
        planes[0, :E_CORE] = ds
        planes[0, E_CORE:] = 1.0                       # pad: harmless d
        vc = vec[s][order]
        planes[1, :E_CORE] = vc[:, 0]
        planes[2, :E_CORE] = vc[:, 1]
        planes[3, :E_CORE] = vc[:, 2]
        muv = mus[ivc]
        planes[4, :E_CORE] = muv[:, 0]
        planes[5, :E_CORE] = muv[:, 1]
        planes[6, :E_CORE] = muv[:, 2]
        muu = mus[iuc]
        planes[7, :E_CORE] = muu[:, 0]
        planes[8, :E_CORE] = muu[:, 1]
        planes[9, :E_CORE] = muu[:, 2]
        planes[10, :E_CORE] = qs[ivc]
        planes[11, :E_CORE] = qs2[iuc]
        bv = bt[ivc]
        for k in range(6):
            planes[12 + k, :E_CORE] = bv[:, k]
        planes[4:, n_live:] = 0.0

        # slot k -> (p = k % P, w = k // P): column-major so ascending d
        # fills tile 0 first.  Per tile: [P, 18, W_t] flattened, tiles
        # concatenated -> [P, 18*W_TOT] fp16.
        pv = planes.reshape(N_PLANES, W_TOT, P)        # [k, w, p]
        chunks = []
        w0 = 0
        for W in TILE_W:
            blk = pv[:, w0:w0 + W, :].transpose(2, 0, 1).reshape(P, N_PLANES * W)
            chunks.append(blk)
            w0 += W
        xi = np.ascontiguousarray(
            np.concatenate(chunks, axis=1).astype(np.float16)
        )
        in_maps.append({"x": xi})
    return in_maps, orders


def _run(inputs, trace=False, tmpdir=None):
    in_maps, orders = _prep_inputs(**inputs)
    nc = _build_module()
    _split_sync_waits(nc)
    res = run_bass_kernel_spmd(
        nc, in_maps, list(range(N_CORES)), trace=trace, tmpdir=tmpdir
    )
    full = np.empty(N_EDGES, dtype=np.float32)
    for c in range(N_CORES):
        o = res.results[c]["out"]                      # [P, W_TOT] fp16
        slots = o.T.reshape(-1)[:E_CORE].astype(np.float32)
        full[c * E_CORE + orders[c]] = slots
    return full, res


def kernel(**inputs):
    full, _ = _run(inputs, trace=False)
    return full


# revision 12
# speedup vs baseline: 1.9526x; 1.0132x over previous
"""Damped electrostatics (charge+dipole+quadrupole, switched) over 3.2M edges
on 8 Trainium2 NeuronCores.

Strategy (data-parallel over edges):
  - Shard the [E]-indexed tensors across the 8 cores (400k edges each).
  - Per-edge u/v atom records are resolved during host-side sharding into
    planar per-edge streams (device indirect-DMA gathers cost ~1.4us per 128
    records on this HW -- far off the roofline; streaming planar operands
    is the only way to feed the DVE at rate).
  - All device math runs in fp16: every DVE tensor_tensor ALU op qualifies
    for the 2x_1p perf mode (2-byte packed operands -> 0.5 cycle/elem) and
    DMA bytes halve.  Tolerance is 2e-2 vs a measured fp32 error of ~3e-6,
    so fp16 (~1e-3 elementwise) has ample margin.  Ln/Exp intermediates
    (L = ln d) stay fp32 on the ACT engine: Exp(-5L) amplifies input error
    5x and fp16 quantization of L would cost ~1% there.
  - Edges are sorted by distance within each core (the slot->edge mapping is
    inverted on unshard).  With ascending d all d<2 edges land in tile 0:
    only that tile evaluates the quintic switch / damped-Coulomb blend
    (exact for d>=2 too, so tile-0 overflow slots are still correct).
    Tiles 1..4 use chi = 1/d via the ACT Ln/Exp tables.  Only the last tile
    needs the d <= CUTOFF mask (largest d sorts there).
  - Constant folding: sqrt(KEHALF) is folded into the per-atom charge/
    dipole/quadrupole tables on the host, so every per-edge u*v product
    carries KEHALF automatically.  The charge-term 2x (from qu2 = 2*qu,
    needed by the dipole term) is cancelled by folding ln(0.5) into the
    Exp biases of r1 and r5.  The quadrupole table is pre-reduced per atom:
    B = sym(Q) - (tr(Q)/3) I with off-diagonals doubled, so the per-edge
    quadrupole contraction is v^T B v (6 products).
  - Vector-engine work is issued as wide block ops over contiguous 3-plane
    groups ([128, 3W] per instruction) to amortize per-instruction
    overhead; tiles are uneven (one 400-wide switch tile, four 700-wide
    fast tiles) so the expensive switch path only covers the columns that
    need it.  GPSIMD is intentionally NOT used: it contends with DVE for
    SBUF ports (measured ~40% slowdown of concurrent DVE ops).
"""

import os
import sys

for _p in ("/opt/trn_rl_repo", "/root/.axon_site/_ro/trn_rl_repo"):
    if os.path.isdir(_p) and _p not in sys.path:
        sys.path.append(_p)

import numpy as np

import concourse.bass as bass
import concourse.mybir as mybir
import concourse.tile as tile
from concourse.bass_utils import run_bass_kernel_spmd

F32 = mybir.dt.float32
F16 = mybir.dt.float16
ALU = mybir.AluOpType
ACT = mybir.ActivationFunctionType

N_CORES = 8
N_ATOMS = 100000
N_EDGES = 3200000
E_CORE = N_EDGES // N_CORES          # 400000
P = 128
W_TOT = 3200                         # 409600 slots >= 400000
TILE_W = [400, 934, 933, 933]        # tile 0 = switch region (d < 2)
N_PLANES = 18
LN_HALF = -0.6931471805599453

CUTOFF = 12.0
KEHALF = 7.199822675975274

_MAX_WAITS = 1  # this walrus build allows only 1 sync wait on some instruction types


def _split_sync_waits(nc):
    """Walrus here fails codegen ("Too many sync wait commands") for any
    instruction carrying more than _MAX_WAITS semaphore waits. Move excess
    waits onto same-engine NOPs inserted immediately before the instruction:
    the sequencer executes waits in program order, so this is equivalent."""
    import bass_rust

    counter = [0]
    for fn in nc.m.functions:
        for bb in fn.blocks:
            insts = list(bb.instructions)
            out = []
            changed = False
            for inst in insts:
                si = inst.sync_info
                waits = list(si.on_wait) if (si and si.on_wait) else []
                if len(waits) > _MAX_WAITS:
                    changed = True
                    head, rest = waits[:-_MAX_WAITS], waits[-_MAX_WAITS:]
                    for i in range(0, len(head), _MAX_WAITS):
                        counter[0] += 1
                        nop = bass_rust.InstNoOp(
                            name=f"I-waitsplit-{counter[0]}", ins=[], outs=[]
                        )
                        nop.engine = inst.engine
                        nop.sync_info = mybir.SyncInfo(
                            on_wait=head[i:i + _MAX_WAITS], on_update=[]
                        )
                        out.append(nop)
                    si.on_wait = rest
                out.append(inst)
            if changed:
                bb.instructions = out


def _register_const(nc, value, dtype=F32):
    t = nc.alloc_sbuf_tensor(f"const-{dtype.name}-{value}", [128, 1], dtype)
    nc.gpsimd.memset(t.ap(), value)
    nc.const_aps.aps[(dtype, value)] = t.ap()


def _build_module():
    nc = bass.Bass()
    _register_const(nc, LN_HALF)
    nc.all_engine_barrier()

    # host packs per tile: [P, sum_t(18*W_t)] fp16, planes contiguous per tile
    total_cols = N_PLANES * W_TOT
    x_in = nc.dram_tensor("x", [P, total_cols], F16, kind="ExternalInput")
    out = nc.dram_tensor("out", [P, W_TOT], F16, kind="ExternalOutput")

    with tile.TileContext(nc) as tc:
        with (
            tc.tile_pool(name="io", bufs=3) as io_pool,
            tc.tile_pool(name="scr", bufs=2) as scr_pool,
        ):
            x_off = 0
            o_off = 0
            for it, W in enumerate(TILE_W):
                slow = it == 0                    # only tile 0 holds d < 2
                masked = it == len(TILE_W) - 1    # only last tile holds d > CUTOFF
                W3 = 3 * W

                # geometry planes (d, v) land first so the ACT chi chain and
                # the v-only DVE work start while the atom block streams in
                xt = io_pool.tile([P, N_PLANES * W], F16, tag="xt")
                nc.sync.dma_start(
                    out=xt[:, 0:4 * W], in_=x_in[:, x_off:x_off + 4 * W]
                )
                nc.sync.dma_start(
                    out=xt[:, 4 * W:N_PLANES * W],
                    in_=x_in[:, x_off + 4 * W:x_off + N_PLANES * W],
                )
                x_off += N_PLANES * W

                def pl(k, n=1):
                    return xt[:, k * W:(k + n) * W]

                d = pl(0)
                vblk = pl(1, 3)
                wblk = pl(4, 3)
                ublk = pl(7, 3)
                qv = pl(10)
                qu2 = pl(11)
                bblk = pl(12, 3)
                cblk = pl(15, 3)
                v0, v1, v2 = pl(1), pl(2), pl(3)

                def scr(tag, w=W, dt=F16):
                    return scr_pool.tile([P, w], dt, tag=tag, name=tag)

                # --- chi powers on the ACT engine ---------------------------
                # r1 = 0.5*chi ; rA = chi^2/d (term1) ; rB = chi^3 ;
                # r5 = 0.5*chi^3/d^2.  Fast path (chi = 1/d): rA == rB.
                r1 = scr("r1")
                r3 = scr("r3")
                r5 = scr("r5")
                if slow:
                    # chi(d) = sw/sqrt(d^2+1) + (1-sw)/d, quintic sw
                    sq = scr("sq", dt=F32)
                    nc.scalar.activation(sq[:], d, ACT.Square)
                    L2 = scr("L2", dt=F32)
                    nc.scalar.activation(L2[:], sq[:], ACT.Ln, bias=1.0)
                    ri = scr("ri")        # 0.5/sqrt(d^2+1)
                    nc.scalar.activation(
                        ri[:], L2[:], ACT.Exp, bias=LN_HALF, scale=-0.5
                    )
                    L = scr("L", dt=F32)
                    nc.scalar.activation(L[:], d, ACT.Ln)
                    r = scr("r")          # 0.5/d
                    nc.scalar.activation(
                        r[:], L[:], ACT.Exp, bias=LN_HALF, scale=-1.0
                    )
                    x = scr("x")          # clip(d/2, 0, 1)
                    nc.vector.tensor_scalar(x[:], d, 0.5, 1.0, ALU.mult, ALU.min)
                    h = scr("h")          # 15 - 6x
                    nc.vector.tensor_scalar(
                        h[:], x[:], -6.0, 15.0, ALU.mult, ALU.add
                    )
                    x2 = scr("x2")
                    nc.scalar.activation(x2[:], x[:], ACT.Square)
                    x3 = scr("x3")
                    nc.vector.tensor_tensor(x3[:], x2[:], x[:], ALU.mult)
                    h2 = scr("h2")        # x*(15-6x)
                    nc.vector.tensor_tensor(h2[:], h[:], x[:], ALU.mult)
                    swm1 = scr("swm1")    # sw - 1 = (h2 - 10)*x^3
                    nc.vector.scalar_tensor_tensor(
                        swm1[:], h2[:], -10.0, x3[:], ALU.add, ALU.mult
                    )
                    rdif = scr("rdif")    # ri - r
                    nc.vector.tensor_tensor(rdif[:], ri[:], r[:], ALU.subtract)
                    # r1 = ri + (sw-1)*(ri-r) = 0.5*chi
                    nc.vector.tensor_tensor(r1[:], swm1[:], rdif[:], ALU.mult)
                    nc.vector.tensor_tensor(r1[:], r1[:], ri[:], ALU.add)
                    c2 = scr("c2")        # chi^2 = (2*r1)^2
                    nc.scalar.activation(c2[:], r1[:], ACT.Square, scale=2.0)
                    rA = scr("rA")        # chi^2/d = c2 * 2 * (0.5/d)
                    nc.vector.scalar_tensor_tensor(
                        rA[:], c2[:], 2.0, r[:], ALU.mult, ALU.mult
                    )
                    # rB = chi^3 = 2*c2*r1
                    nc.vector.scalar_tensor_tensor(
                        r3[:], c2[:], 2.0, r1[:], ALU.mult, ALU.mult
                    )
                    rr2 = scr("rr2")      # 1/d^2 = (2*r)^2
                    nc.scalar.activation(rr2[:], r[:], ACT.Square, scale=2.0)
                    # r5 = 0.5*chi^3/d^2
                    nc.vector.scalar_tensor_tensor(
                        r5[:], r3[:], 0.5, rr2[:], ALU.mult, ALU.mult
                    )
                else:
                    # d >= 2 -> sw == 0 -> chi = 1/d exactly (ACT Ln/Exp
                    # tables, ~1.3e-4 rel; tolerance is 2e-2)
                    L = scr("L", dt=F32)
                    nc.scalar.activation(L[:], d, ACT.Ln)
                    nc.scalar.activation(
                        r1[:], L[:], ACT.Exp, bias=LN_HALF, scale=-1.0
                    )
                    nc.scalar.activation(r3[:], L[:], ACT.Exp, scale=-3.0)
                    nc.scalar.activation(
                        r5[:], L[:], ACT.Exp, bias=LN_HALF, scale=-5.0
                    )

                # --- dot products as 3-plane block ops ----------------------
                # liveness-driven in-place: pm first (consumes u,w as pure
                # reads), then the v.w / v.u products overwrite w / u blocks
                pm = scr("pm", W3)        # mu_u .* mu_v
                nc.vector.tensor_tensor(pm[:], ublk, wblk, ALU.mult)
                nc.vector.tensor_tensor(wblk, vblk, wblk, ALU.mult)  # v.*mu_v
                nc.vector.tensor_tensor(ublk, vblk, ublk, ALU.mult)  # v.*mu_u
                sv = wblk[:, 0:W]         # KE * (v . mu_v)
                nc.vector.tensor_tensor(sv, wblk[:, 0:W], wblk[:, W:2 * W], ALU.add)
                nc.vector.tensor_tensor(sv, sv, wblk[:, 2 * W:W3], ALU.add)
                su = ublk[:, 0:W]         # v . mu_u (sqrt(KE) scale)
                nc.vector.tensor_tensor(su, ublk[:, 0:W], ublk[:, W:2 * W], ALU.add)
                nc.vector.tensor_tensor(su, su, ublk[:, 2 * W:W3], ALU.add)
                M = pm[:, 0:W]            # KE * (mu_u . mu_v)
                nc.vector.tensor_tensor(M, pm[:, 0:W], pm[:, W:2 * W], ALU.add)
                nc.vector.tensor_tensor(M, M, pm[:, 2 * W:W3], ALU.add)

                # --- quadrupole form wq = v^T B v ---------------------------
                vsq = scr("vsq", W3)
                nc.scalar.activation(vsq[:], vblk, ACT.Square)
                vp = scr("vp", W3)        # v0v1 | v0v2 | v1v2
                nc.vector.tensor_tensor(vp[:, 0:W], v0, v1, ALU.mult)
                nc.vector.tensor_tensor(vp[:, W:2 * W], v0, v2, ALU.mult)
                nc.vector.tensor_tensor(vp[:, 2 * W:W3], v1, v2, ALU.mult)
                nc.vector.tensor_tensor(bblk, vsq[:], bblk, ALU.mult)
                nc.vector.tensor_tensor(cblk, vp[:], cblk, ALU.mult)
                nc.vector.tensor_tensor(cblk, bblk, cblk, ALU.add)
                wq = cblk[:, 0:W]
                nc.vector.tensor_tensor(wq, cblk[:, 0:W], cblk[:, W:2 * W], ALU.add)
                nc.vector.tensor_tensor(wq, wq, cblk[:, 2 * W:W3], ALU.add)

                # --- assemble ----------------------------------------------
                # E = cq*r1 + (qu2*sv)*rA + M*rB + (qu2*wq - 6 sv su)*r5
                # (d > CUTOFF edges have multipole planes zeroed on the host,
                # so their E is exactly 0 without a mask op)
                e = qv                    # in-place: qv dead after first op
                nc.vector.tensor_tensor(e, qu2, qv, ALU.mult)
                nc.vector.tensor_tensor(e, e, r1[:], ALU.mult)
                t = scr("t")
                nc.vector.tensor_tensor(t[:], qu2, sv, ALU.mult)
                if slow:
                    nc.vector.tensor_tensor(t[:], t[:], rA[:], ALU.mult)
                    nc.vector.tensor_tensor(e, e, t[:], ALU.add)
                    nc.vector.tensor_tensor(t[:], M, r3[:], ALU.mult)
                    nc.vector.tensor_tensor(e, e, t[:], ALU.add)
                else:
                    nc.vector.tensor_tensor(t[:], t[:], M, ALU.add)
                    nc.vector.tensor_tensor(t[:], t[:], r3[:], ALU.mult)
                    nc.vector.tensor_tensor(e, e, t[:], ALU.add)
                nc.vector.tensor_tensor(wq, wq, qu2, ALU.mult)
                p = vp[:, 0:W]            # vp dead after the cblk product
                nc.vector.tensor_tensor(p, sv, su, ALU.mult)
                nc.vector.scalar_tensor_tensor(
                    p, p, -6.0, wq, ALU.mult, ALU.add
                )
                nc.vector.tensor_tensor(p, p, r5[:], ALU.mult)

                res = io_pool.tile([P, W], F16, tag="res")
                nc.vector.tensor_tensor(res[:], e, p, ALU.add)

                nc.sync.dma_start(out=out[:, o_off:o_off + W], in_=res[:])
                o_off += W

    return nc


def _prep_inputs(distances_uv, vectors_uv, atomic_charges, atomic_dipoles,
                 atomic_quadrupoles, idx_u, idx_v):
    d = np.ascontiguousarray(np.asarray(distances_uv, dtype=np.float32))
    vec = np.ascontiguousarray(np.asarray(vectors_uv, dtype=np.float32))
    q = np.asarray(atomic_charges, dtype=np.float32)
    mu = np.asarray(atomic_dipoles, dtype=np.float32)
    Q = np.asarray(atomic_quadrupoles, dtype=np.float32)
    iu = np.asarray(idx_u, dtype=np.int64)
    iv = np.asarray(idx_v, dtype=np.int64)

    rke = np.float32(np.sqrt(KEHALF))
    qs = rke * q                      # sqrt(KE) * q
    qs2 = 2.0 * qs                    # 2 sqrt(KE) * q
    mus = rke * mu                    # sqrt(KE) * mu

    # traceless symmetrized quadrupole, off-diagonals doubled, sqrt(KE) scaled
    B = 0.5 * (Q + np.swapaxes(Q, 1, 2))
    tr3 = (np.trace(Q, axis1=1, axis2=2) / 3.0).astype(np.float32)
    bt = np.empty((N_ATOMS, 6), dtype=np.float32)
    bt[:, 0] = rke * (B[:, 0, 0] - tr3)
    bt[:, 1] = rke * (B[:, 1, 1] - tr3)
    bt[:, 2] = rke * (B[:, 2, 2] - tr3)
    bt[:, 3] = rke * 2.0 * B[:, 0, 1]
    bt[:, 4] = rke * 2.0 * B[:, 0, 2]
    bt[:, 5] = rke * 2.0 * B[:, 1, 2]

    n_slots = P * W_TOT
    in_maps = []
    orders = []
    for c in range(N_CORES):
        s = slice(c * E_CORE, (c + 1) * E_CORE)
        dc = d[s]
        order = np.argsort(dc, kind="stable")
        orders.append(order)
        n_lt2 = int((dc < 2.0).sum())
        assert n_lt2 <= P * TILE_W[0], (
            f"core {c}: {n_lt2} edges with d<2 exceed the switch tile"
        )

        iuc = iu[s][order]
        ivc = iv[s][order]
        ds = dc[order]
        # edges beyond the cutoff sort to the tail: zero their multipole
        # planes so every term vanishes exactly (replaces the device mask)
        n_live = int(np.searchsorted(ds, np.float32(CUTOFF), side="right"))
        planes = np.zeros((N_PLANES, n_slots), dtype=np.float32)
        planes[0, :E_CORE] = ds
        planes[0, E_CORE:] = 1.0                       # pad: harmless d
        vc = vec[s][order]
        planes[1, :E_CORE] = vc[:, 0]
        planes[2, :E_CORE] = vc[:, 1]
        planes[3, :E_CORE] = vc[:, 2]
        muv = mus[ivc]
        planes[4, :E_CORE] = muv[:, 0]
        planes[5, :E_CORE] = muv[:, 1]
        planes[6, :E_CORE] = muv[:, 2]
        muu = mus[iuc]
        planes[7, :E_CORE] = muu[:, 0]
        planes[8, :E_CORE] = muu[:, 1]
        planes[9, :E_CORE] = muu[:, 2]
        planes[10, :E_CORE] = qs[ivc]
        planes[11, :E_CORE] = qs2[iuc]
        bv = bt[ivc]
        for k in range(6):
            planes[12 + k, :E_CORE] = bv[:, k]
        planes[4:, n_live:] = 0.0

        # slot k -> (p = k % P, w = k // P): column-major so ascending d
        # fills tile 0 first.  Per tile: [P, 18, W_t] flattened, tiles
        # concatenated -> [P, 18*W_TOT] fp16.
        pv = planes.reshape(N_PLANES, W_TOT, P)        # [k, w, p]
        chunks = []
        w0 = 0
        for W in TILE_W:
            blk = pv[:, w0:w0 + W, :].transpose(2, 0, 1).reshape(P, N_PLANES * W)
            chunks.append(blk)
            w0 += W
        xi = np.ascontiguousarray(
            np.concatenate(chunks, axis=1).astype(np.float16)
        )
        in_maps.append({"x": xi})
    return in_maps, orders


def _run(inputs, trace=False, tmpdir=None):
    in_maps, orders = _prep_inputs(**inputs)
    nc = _build_module()
    _split_sync_waits(nc)
    res = run_bass_kernel_spmd(
        nc, in_maps, list(range(N_CORES)), trace=trace, tmpdir=tmpdir
    )
    full = np.empty(N_EDGES, dtype=np.float32)
    for c in range(N_CORES):
        o = res.results[c]["out"]                      # [P, W_TOT] fp16
        slots = o.T.reshape(-1)[:E_CORE].astype(np.float32)
        full[c * E_CORE + orders[c]] = slots
    return full, res


def kernel(**inputs):
    full, _ = _run(inputs, trace=False)
    return full
